# revision 32
# baseline (speedup 1.0000x reference)
"""Bidirectional LSTM over embedded event ids — Trainium2 Bass kernel.

Problem shapes (hardcoded): ids [32,64,256] int32, embed [6000,64],
per-direction LSTM E=H=64, output [32,64,256,128] f32.

Strategy: pure data parallel over the flattened B*S=2048 sequence axis
(256 sequences per core on 8 cores).  The per-core 256 sequences are split
into 3 independent pipeline chains (78/90/88) so each chain's serial
per-step dependency path (h-matmuls -> sigmoid -> cell update -> tanh -> h)
hides under the other chains' engine work.  Both directions of one chain
share every instruction.

Per (chain, step), n = chain's sequence count, all tiles dir-stacked
(dir f rows 0:64, dir b rows 64:128), blocks [i | f | o | g] of n cols:
  z   [128,4n] PSUM f32 <- 8 x-matmuls (ctr 64, run a step ahead, PSUM
                           accumulation start) + 8 h-matmuls (ctr 64,
                           accumulate; rhs read straight from the stacked
                           h tile -- no rhs assembly on the critical path)
  zs  [128,4n] f32      <- ONE Sigmoid (g-gate weights pre-scaled by 2:
                           tanh(z) = 2*sig(2z)-1; zs must stay f32 -- a
                           bf16 s_g destroys g's precision near 0)
  t2  = s_f * c           tensor_tensor (DVE for chain 0, GPSIMD for
                           chains 1-2 to break the DVE serialization)
  t1  = (s_g-0.5)*s_i     fused scalar_tensor_tensor
  c   = t1 + t2           c stored as c/2; the x2 folds into Tanh's scale
  tc  = tanh(2*c)         ONE [128,n] Tanh
  h   = s_o * tc          ONE tensor_tensor, bf16, into the shared h tile;
                           the output DMA reads the same bytes (one in-DMA
                           + one out-DMA per step).

The backward direction reads x time-reversed via host-side layout.
bf16: matmul inputs (x, h, weights) and the output; f32: PSUM, gate
sigmoids, cell state.
"""

import numpy as np

EMIT_ORDER = 0
ZSB = 3
TMB = 6
PSUM_C0 = False

B, S, L, E, H, V = 32, 64, 256, 64, 64, 6000
NCORES = 8
NSEQ = B * S
NC_ = NSEQ // NCORES      # 256 sequences per core
GATES = 4 * H             # 256
KDIM = E + H              # 128

SPLITS = (78, 90, 88)     # sequences per chain (sum = NC_)

_CACHE = {}


def _build(l_steps, splits=SPLITS, prefetch=4, c_dtype="f32",
           tail_on=("vector", "t2pool", "t2pool"),
           h_on=("vector", "vector", "vector")):
    import concourse.bacc as bacc
    import concourse.tile as tile
    from concourse import mybir

    dt = mybir.dt
    AF = mybir.ActivationFunctionType
    OP = mybir.AluOpType

    K = len(splits)
    offs = [2 * sum(splits[:c]) for c in range(K)]   # x col offset per chain
    hoffs = [sum(splits[:c]) for c in range(K)]      # h col offset per chain
    W = 2 * sum(splits)                              # 512 x cols
    HW_ = sum(splits)                                # 256 h cols

    nc = bacc.Bacc("TRN2", num_devices=NCORES, debug=False)
    x_d = nc.dram_tensor("x", (E, l_steps, W), dt.bfloat16,
                         kind="ExternalInput")
    w_d = {d: nc.dram_tensor(f"w_{d}", (KDIM, GATES), dt.bfloat16,
                             kind="ExternalInput") for d in ("f", "b")}
    o_d = nc.dram_tensor("o", (KDIM, l_steps, HW_), dt.bfloat16,
                         kind="ExternalOutput")

    cdt = dt.float32 if c_dtype == "f32" else dt.bfloat16

    with tile.TileContext(nc) as tc:
        with (
            tc.tile_pool(name="singles", bufs=1) as singles,
            tc.tile_pool(name="xt", bufs=prefetch + 3) as xt_pool,
            tc.tile_pool(name="ht", bufs=4) as ht_pool,
            tc.tile_pool(name="zs", bufs=ZSB * K) as zs_pool,
            tc.tile_pool(name="tmp", bufs=TMB * K) as tmp_pool,
        ):
            psum_pools = []
            import contextlib
            with contextlib.ExitStack() as stack:
                for c in range(K):
                    psum_pools.append(stack.enter_context(
                        tc.tile_pool(name=f"psum{c}", bufs=2, space="PSUM")))
                psum_singles = stack.enter_context(
                    tc.tile_pool(name="psum_s", bufs=1, space="PSUM")) \
                    if PSUM_C0 else None
                _body(nc, tc, dt, AF, OP, K, splits, offs, hoffs, W, HW_,
                      l_steps, prefetch, cdt, singles, xt_pool, ht_pool,
                      zs_pool, tmp_pool, psum_pools, x_d, w_d, o_d, tail_on,
                      h_on, psum_singles)

    nc.compile()
    return nc


def _body(nc, tc, dt, AF, OP, K, splits, offs, hoffs, W, HW_, l_steps,
          prefetch, cdt, singles, xt_pool, ht_pool, zs_pool, tmp_pool,
          psum_pools, x_d, w_d, o_d, tail_on=("vector",) * 8,
          h_on=("vector",) * 8, psum_singles=None):
    UP, LO = slice(0, 64), slice(64, 128)

    # x-projection weights at base partition 0 (match x tiles); the
    # h-projection weights live where their rhs lives: Wh_f at partitions
    # 0:64 (h_f rows), Wh_b at 64:128 (h_b rows) -- matmul requires lhsT and
    # rhs to share the base partition.
    w_t = {}
    wh_t = {}
    for d in ("f", "b"):
        w_t[d] = singles.tile([E, GATES], dt.bfloat16,
                              name=f"w_{d}", tag=f"w_{d}")
        nc.sync.dma_start(out=w_t[d][:, :], in_=w_d[d].ap()[0:E, :])
    wh_t["f"] = singles.tile([E, GATES], dt.bfloat16, name="wh_f", tag="wh_f")
    nc.sync.dma_start(out=wh_t["f"][:, :], in_=w_d["f"].ap()[E:KDIM, :])
    whb = singles.tile([KDIM, GATES], dt.bfloat16, name="wh_b", tag="wh_b")
    nc.sync.dma_start(out=whb[LO, :], in_=w_d["b"].ap()[E:KDIM, :])
    wh_t["b"] = whb
    # chain 0's cell state + tanh output can live in PSUM (its tail is
    # all-DVE, and DVE may have one PSUM operand per op): cuts the ACT
    # access latency on the wrap-critical tanh0 -> h0 path.
    c_t = []
    tc0_t = None
    for c in range(K):
        pool = psum_singles if (psum_singles is not None and c == 0) \
            else singles
        ct = pool.tile([KDIM, splits[c]], cdt, name=f"c{c}", tag=f"c{c}")
        nc.vector.memset(ct[:, :], 0.0)
        c_t.append(ct)
    if psum_singles is not None:
        tc0_t = psum_singles.tile([KDIM, splits[0]], dt.float32,
                                  name="tc0s", tag="tc0s")

    xt_tiles = {}
    ht_tiles = {}
    z_tiles = {}

    def new_x(t):
        tl = xt_pool.tile([E, W], dt.bfloat16, name="xt", tag="xt")
        xt_tiles[t] = tl
        nc.sync.dma_start(out=tl[:, :], in_=x_d.ap()[:, t, :])

    # per-chain z layout [128, 4n]: blocks [i | f | o | g] of n cols, each
    # dir-stacked (f rows 0:64, b rows 64:128).  Keras gate cols in w:
    # i 0:64, f 64:128, g 128:192 (x2), o 192:256.
    GCOL = {"i": slice(0, 64), "f": slice(64, 128),
            "o": slice(192, 256), "g": slice(128, 192)}
    ORDER = ("i", "f", "o", "g")

    def x_mms(c, t):
        # x-projection, runs ahead of the recurrence (start of the psum
        # accumulation group; t=0 also closes it -- no h contribution)
        n = splits[c]
        o = offs[c]
        z = psum_pools[c].tile([128, 4 * n], dt.float32,
                               name=f"z{c}", tag=f"z{c}")
        z_tiles[(c, t)] = z
        xf = xt_tiles[t][:, o:o + n]
        xb = xt_tiles[t][:, o + n:o + 2 * n]
        last = t == 0
        for k, gate in enumerate(ORDER):
            nc.tensor.matmul(z[UP, k * n:(k + 1) * n],
                             w_t["f"][:, GCOL[gate]], xf,
                             start=True, stop=last)
            nc.tensor.matmul(z[LO, k * n:(k + 1) * n],
                             w_t["b"][:, GCOL[gate]], xb,
                             start=True, stop=last)
        return z

    def h_mms(c, t):
        # recurrent part: contraction 64 over h(t-1), read straight from the
        # dir-stacked h tile (no rhs assembly on the critical path)
        n = splits[c]
        ho = hoffs[c]
        z = z_tiles.pop((c, t))
        hprev = ht_tiles[t - 1]
        hf = hprev[UP, ho:ho + n]
        hb = hprev[LO, ho:ho + n]
        for k, gate in enumerate(ORDER):
            nc.tensor.matmul(z[UP, k * n:(k + 1) * n],
                             wh_t["f"][:, GCOL[gate]], hf,
                             start=False, stop=True)
        for k, gate in enumerate(ORDER):
            nc.tensor.matmul(z[LO, k * n:(k + 1) * n],
                             wh_t["b"][LO, GCOL[gate]], hb,
                             start=False, stop=True)
        return z

    def sig_chain(c, z):
        n = splits[c]
        zs = zs_pool.tile([128, 4 * n], dt.float32,
                          name=f"zs{c}", tag=f"zs{c}")
        nc.scalar.activation(zs[:, :], z[:, :], AF.Sigmoid)
        return zs

    def tail_chain(c, zs):
        # c_new = s_f*c + 2*(s_i*(s_g-0.5)); s_* stay f32 (a bf16 s_g makes
        # 2*s_g-1 lose all precision near g=0).  All [128, n] dir-stacked.
        n = splits[c]
        s_i = zs[:, 0:n]
        s_f = zs[:, n:2 * n]
        s_g = zs[:, 3 * n:4 * n]
        # the cell state is stored as c/2 so the combine is a plain add;
        # tanh reads it with scale=2 (free input affine on ACT)
        mode = tail_on[c % len(tail_on)]
        t2e = nc.gpsimd if mode in ("pool", "t2pool", "t2addpool") else nc.vector
        t1e = nc.gpsimd if mode == "pool" else nc.vector
        adde = nc.gpsimd if mode in ("pool", "addpool", "t2addpool") else nc.vector
        t2 = tmp_pool.tile([KDIM, n], dt.float32, name=f"t2{c}", tag=f"t2{c}")
        t2e.tensor_mul(t2[:, :], s_f, c_t[c][:, :])
        t1 = tmp_pool.tile([KDIM, n], dt.float32, name=f"t1{c}", tag=f"t1{c}")
        t1e.scalar_tensor_tensor(
            out=t1[:, :], in0=s_g, scalar=0.5, in1=s_i,
            op0=OP.subtract, op1=OP.mult)
        adde.tensor_add(c_t[c][:, :], t1[:, :], t2[:, :])

    def tanh_chain(c):
        n = splits[c]
        if c == 0 and tc0_t is not None:
            tch = tc0_t
        else:
            tch = tmp_pool.tile([KDIM, n], dt.float32,
                                name=f"tc{c}", tag=f"tc{c}")
        nc.scalar.activation(tch[:, :], c_t[c][:, :], AF.Tanh, scale=2.0)
        return tch

    def h_chain(c, zs, tch, t):
        n = splits[c]
        ho = hoffs[c]
        mode = h_on[c % len(h_on)]
        eng = nc.gpsimd if mode == "pool" else nc.vector
        if mode == "split":
            # f-half first: unlocks the 4 f-dir h-matmuls of step t+1 early
            nc.vector.tensor_mul(ht_tiles[t][UP, ho:ho + n],
                                 zs[UP, 2 * n:3 * n], tch[UP, :])
            nc.vector.tensor_mul(ht_tiles[t][LO, ho:ho + n],
                                 zs[LO, 2 * n:3 * n], tch[LO, :])
        else:
            eng.tensor_mul(ht_tiles[t][:, ho:ho + n], zs[:, 2 * n:3 * n],
                           tch[:, :])

    for t0 in range(min(prefetch, l_steps)):
        new_x(t0)
    for c in range(K):
        x_mms(c, 0)

    for t in range(l_steps):
        if t + prefetch < l_steps:
            new_x(t + prefetch)
        ht_tiles[t] = ht_pool.tile([KDIM, HW_], dt.bfloat16,
                                   name="ht", tag="ht")
        zs_l = []
        if EMIT_ORDER == 0:
            for c in range(K):
                z = h_mms(c, t) if t > 0 else z_tiles.pop((c, 0))
                zs_l.append(sig_chain(c, z))
                tail_chain(c, zs_l[c])
            for c in range(K):
                tch = tanh_chain(c)
                h_chain(c, zs_l[c], tch, t)
        else:
            # tanh_c emitted after sig_{c+1}: ACT order s0 s1 t0 s2 t1 t2
            for c in range(K):
                z = h_mms(c, t) if t > 0 else z_tiles.pop((c, 0))
                zs_l.append(sig_chain(c, z))
                tail_chain(c, zs_l[c])
                if c >= 1:
                    tch = tanh_chain(c - 1)
                    h_chain(c - 1, zs_l[c - 1], tch, t)
            for c in range(K - 1, K):
                tch = tanh_chain(c)
                h_chain(c, zs_l[c], tch, t)
        if t + 1 < l_steps:
            for c in range(K):
                x_mms(c, t + 1)
        nc.sync.dma_start(out=o_d.ap()[:, t, :], in_=ht_tiles[t][:, :])
        if t >= 2:
            del ht_tiles[t - 2]
        if t in xt_tiles:
            del xt_tiles[t]


def _get_nc(l_steps, **kw):
    key = (l_steps, tuple(sorted(kw.items())))
    if key not in _CACHE:
        _CACHE[key] = _build(l_steps, **kw)
    return _CACHE[key]


def _prep_w(Wk, Wr):
    """[128, 256] contiguous: rows = [x-proj; h-proj], Keras gate col order
    i,f,g,o kept as-is; g cols pre-scaled by 2 (tanh via sigmoid)."""
    Wcat = np.concatenate([np.asarray(Wk, np.float32),
                           np.asarray(Wr, np.float32)], axis=0).copy()
    Wcat[:, 128:192] *= 2.0
    return Wcat


def kernel(ids, embed_table, Wk_f, Wr_f, b_f, Wk_b, Wr_b, b_b):
    import ml_dtypes
    from concourse import bass_utils

    bf16 = ml_dtypes.bfloat16
    assert not np.any(np.asarray(b_f)) and not np.any(np.asarray(b_b)), \
        "nonzero LSTM bias not supported by this kernel build"

    ids = np.asarray(ids)
    emb = np.asarray(embed_table, dtype=np.float32)
    wf = _prep_w(Wk_f, Wr_f).astype(bf16)
    wb = _prep_w(Wk_b, Wr_b).astype(bf16)

    nc = _get_nc(L)
    splits = SPLITS
    offs = [2 * sum(splits[:c]) for c in range(len(splits))]
    W = 2 * sum(splits)

    ids2 = ids.reshape(NSEQ, L)
    in_maps = []
    for m in range(NCORES):
        idc = ids2[m * NC_:(m + 1) * NC_]            # [NC_, L]
        xc = emb[idc]                                # [NC_, L, E] f32
        xT = xc.transpose(2, 1, 0)                   # [E, L, NC_] view
        xcat = np.empty((E, L, W), dtype=np.float32)
        s0 = 0
        for c, n in enumerate(splits):
            o = offs[c]
            xcat[:, :, o:o + n] = xT[:, :, s0:s0 + n]
            xcat[:, :, o + n:o + 2 * n] = xT[:, ::-1, s0:s0 + n]
            s0 += n
        im = {"x": xcat.astype(bf16), "w_f": wf, "w_b": wb}
        in_maps.append(im)

    res = bass_utils.run_bass_kernel_spmd(nc, in_maps,
                                          core_ids=list(range(NCORES)))

    hoffs = [sum(splits[:c]) for c in range(len(splits))]
    out = np.empty((NSEQ, L, 2 * H), dtype=np.float32)
    for m in range(NCORES):
        o_arr = np.asarray(res.results[m]["o"], dtype=np.float32)  # [128,L,256]
        s0 = 0
        for c, n in enumerate(splits):
            ho = hoffs[c]
            sl = slice(m * NC_ + s0, m * NC_ + s0 + n)
            hf = o_arr[0:H, :, ho:ho + n]              # [H, L, n]
            hb = o_arr[H:2 * H, ::-1, ho:ho + n]       # iteration -> time
            out[sl, :, 0:H] = hf.transpose(2, 1, 0)
            out[sl, :, H:2 * H] = hb.transpose(2, 1, 0)
            s0 += n
    return out.reshape(B, S, L, 2 * H)


# revision 34
# speedup vs baseline: 1.0019x; 1.0019x over previous
"""Bidirectional LSTM over embedded event ids — Trainium2 Bass kernel.

Problem shapes (hardcoded): ids [32,64,256] int32, embed [6000,64],
per-direction LSTM E=H=64, output [32,64,256,128] f32.

Strategy: pure data parallel over the flattened B*S=2048 sequence axis
(256 sequences per core on 8 cores).  The per-core 256 sequences are split
into 3 independent pipeline chains (78/90/88) so each chain's serial
per-step dependency path (h-matmuls -> sigmoid -> cell update -> tanh -> h)
hides under the other chains' engine work.  Both directions of one chain
share every instruction.

Per (chain, step), n = chain's sequence count, all tiles dir-stacked
(dir f rows 0:64, dir b rows 64:128), blocks [i | f | o | g] of n cols:
  z   [128,4n] PSUM f32 <- 8 x-matmuls (ctr 64, run a step ahead, PSUM
                           accumulation start) + 8 h-matmuls (ctr 64,
                           accumulate; rhs read straight from the stacked
                           h tile -- no rhs assembly on the critical path)
  zs  [128,4n] f32      <- ONE Sigmoid (g-gate weights pre-scaled by 2:
                           tanh(z) = 2*sig(2z)-1; zs must stay f32 -- a
                           bf16 s_g destroys g's precision near 0)
  t2  = s_f * c           tensor_tensor (DVE for chain 0, GPSIMD for
                           chains 1-2 to break the DVE serialization)
  t1  = (s_g-0.5)*s_i     fused scalar_tensor_tensor
  c   = t1 + t2           c stored as c/2; the x2 folds into Tanh's scale
  tc  = tanh(2*c)         ONE [128,n] Tanh
  h   = s_o * tc          ONE tensor_tensor, bf16, into the shared h tile;
                           the output DMA reads the same bytes (one in-DMA
                           + one out-DMA per step).

The backward direction reads x time-reversed via host-side layout.
bf16: matmul inputs (x, h, weights) and the output; f32: PSUM, gate
sigmoids, cell state.
"""

import numpy as np

EMIT_ORDER = 0
ZSB = 3
TMB = 6
PSUM_C0 = False

B, S, L, E, H, V = 32, 64, 256, 64, 64, 6000
NCORES = 8
NSEQ = B * S
NC_ = NSEQ // NCORES      # 256 sequences per core
GATES = 4 * H             # 256
KDIM = E + H              # 128

SPLITS = (78, 90, 88)     # sequences per chain (sum = NC_)

_CACHE = {}


def _build(l_steps, splits=SPLITS, prefetch=4, c_dtype="f32",
           tail_on=("vector", "t2pool", "t2pool"),
           h_on=("vector", "vector", "vector")):
    import concourse.bacc as bacc
    import concourse.tile as tile
    from concourse import mybir

    dt = mybir.dt
    AF = mybir.ActivationFunctionType
    OP = mybir.AluOpType

    K = len(splits)
    offs = [2 * sum(splits[:c]) for c in range(K)]   # x col offset per chain
    hoffs = [sum(splits[:c]) for c in range(K)]      # h col offset per chain
    W = 2 * sum(splits)                              # 512 x cols
    HW_ = sum(splits)                                # 256 h cols

    nc = bacc.Bacc("TRN2", num_devices=NCORES, debug=False)
    x_d = nc.dram_tensor("x", (E, l_steps, W), dt.bfloat16,
                         kind="ExternalInput")
    wx_d = nc.dram_tensor("wx", (E, 2 * GATES), dt.bfloat16,
                          kind="ExternalInput")
    wh_d = nc.dram_tensor("wh", (KDIM, GATES), dt.bfloat16,
                          kind="ExternalInput")
    o_d = nc.dram_tensor("o", (KDIM, l_steps, HW_), dt.bfloat16,
                         kind="ExternalOutput")

    cdt = dt.float32 if c_dtype == "f32" else dt.bfloat16

    with tile.TileContext(nc) as tc:
        with (
            tc.tile_pool(name="singles", bufs=1) as singles,
            tc.tile_pool(name="xt", bufs=prefetch + 3) as xt_pool,
            tc.tile_pool(name="ht", bufs=4) as ht_pool,
            tc.tile_pool(name="zs", bufs=ZSB * K) as zs_pool,
            tc.tile_pool(name="tmp", bufs=TMB * K) as tmp_pool,
        ):
            psum_pools = []
            import contextlib
            with contextlib.ExitStack() as stack:
                for c in range(K):
                    psum_pools.append(stack.enter_context(
                        tc.tile_pool(name=f"psum{c}", bufs=2, space="PSUM")))
                psum_singles = stack.enter_context(
                    tc.tile_pool(name="psum_s", bufs=1, space="PSUM")) \
                    if PSUM_C0 else None
                _body(nc, tc, dt, AF, OP, K, splits, offs, hoffs, W, HW_,
                      l_steps, prefetch, cdt, singles, xt_pool, ht_pool,
                      zs_pool, tmp_pool, psum_pools, x_d, wx_d, wh_d, o_d,
                      tail_on, h_on, psum_singles)

    nc.compile()
    return nc


def _body(nc, tc, dt, AF, OP, K, splits, offs, hoffs, W, HW_, l_steps,
          prefetch, cdt, singles, xt_pool, ht_pool, zs_pool, tmp_pool,
          psum_pools, x_d, wx_d, wh_d, o_d, tail_on=("vector",) * 8,
          h_on=("vector",) * 8, psum_singles=None):
    UP, LO = slice(0, 64), slice(64, 128)

    # weights host-packed into two tensors (2 DMAs instead of 4): wx holds
    # both dirs' x-projections side by side at base partition 0 (match x
    # tiles); wh holds Wh_f at partitions 0:64 and Wh_b at 64:128, where
    # their rhs (the stacked h tile) lives -- matmul requires lhsT and rhs
    # to share the base partition.
    wx_t = singles.tile([E, 2 * GATES], dt.bfloat16, name="wx", tag="wx")
    nc.sync.dma_start(out=wx_t[:, :], in_=wx_d.ap())
    wh_t = singles.tile([KDIM, GATES], dt.bfloat16, name="wh", tag="wh")
    nc.sync.dma_start(out=wh_t[:, :], in_=wh_d.ap())
    # chain 0's cell state + tanh output can live in PSUM (its tail is
    # all-DVE, and DVE may have one PSUM operand per op): cuts the ACT
    # access latency on the wrap-critical tanh0 -> h0 path.
    c_t = []
    tc0_t = None
    for c in range(K):
        pool = psum_singles if (psum_singles is not None and c == 0) \
            else singles
        ct = pool.tile([KDIM, splits[c]], cdt, name=f"c{c}", tag=f"c{c}")
        nc.vector.memset(ct[:, :], 0.0)
        c_t.append(ct)
    if psum_singles is not None:
        tc0_t = psum_singles.tile([KDIM, splits[0]], dt.float32,
                                  name="tc0s", tag="tc0s")

    xt_tiles = {}
    ht_tiles = {}
    z_tiles = {}

    def new_x(t):
        tl = xt_pool.tile([E, W], dt.bfloat16, name="xt", tag="xt")
        xt_tiles[t] = tl
        nc.sync.dma_start(out=tl[:, :], in_=x_d.ap()[:, t, :])

    # per-chain z layout [128, 4n]: blocks [i | f | o | g] of n cols, each
    # dir-stacked (f rows 0:64, b rows 64:128).  Keras gate cols in w:
    # i 0:64, f 64:128, g 128:192 (x2), o 192:256.
    GCOL = {"i": slice(0, 64), "f": slice(64, 128),
            "o": slice(192, 256), "g": slice(128, 192)}
    ORDER = ("i", "f", "o", "g")

    def x_mms(c, t):
        # x-projection, runs ahead of the recurrence (start of the psum
        # accumulation group; t=0 also closes it -- no h contribution)
        n = splits[c]
        o = offs[c]
        z = psum_pools[c].tile([128, 4 * n], dt.float32,
                               name=f"z{c}", tag=f"z{c}")
        z_tiles[(c, t)] = z
        xf = xt_tiles[t][:, o:o + n]
        xb = xt_tiles[t][:, o + n:o + 2 * n]
        last = t == 0
        for k, gate in enumerate(ORDER):
            g = GCOL[gate]
            nc.tensor.matmul(z[UP, k * n:(k + 1) * n],
                             wx_t[:, g], xf, start=True, stop=last)
            nc.tensor.matmul(z[LO, k * n:(k + 1) * n],
                             wx_t[:, g.start + GATES:g.stop + GATES], xb,
                             start=True, stop=last)
        return z

    def h_mms(c, t):
        # recurrent part: contraction 64 over h(t-1), read straight from the
        # dir-stacked h tile (no rhs assembly on the critical path)
        n = splits[c]
        ho = hoffs[c]
        z = z_tiles.pop((c, t))
        hprev = ht_tiles[t - 1]
        hf = hprev[UP, ho:ho + n]
        hb = hprev[LO, ho:ho + n]
        for k, gate in enumerate(ORDER):
            nc.tensor.matmul(z[UP, k * n:(k + 1) * n],
                             wh_t[UP, GCOL[gate]], hf,
                             start=False, stop=True)
        for k, gate in enumerate(ORDER):
            nc.tensor.matmul(z[LO, k * n:(k + 1) * n],
                             wh_t[LO, GCOL[gate]], hb,
                             start=False, stop=True)
        return z

    def sig_chain(c, z):
        n = splits[c]
        zs = zs_pool.tile([128, 4 * n], dt.float32,
                          name=f"zs{c}", tag=f"zs{c}")
        nc.scalar.activation(zs[:, :], z[:, :], AF.Sigmoid)
        return zs

    def tail_chain(c, zs):
        # c_new = s_f*c + 2*(s_i*(s_g-0.5)); s_* stay f32 (a bf16 s_g makes
        # 2*s_g-1 lose all precision near g=0).  All [128, n] dir-stacked.
        n = splits[c]
        s_i = zs[:, 0:n]
        s_f = zs[:, n:2 * n]
        s_g = zs[:, 3 * n:4 * n]
        # the cell state is stored as c/2 so the combine is a plain add;
        # tanh reads it with scale=2 (free input affine on ACT)
        mode = tail_on[c % len(tail_on)]
        t2e = nc.gpsimd if mode in ("pool", "t2pool", "t2addpool") else nc.vector
        t1e = nc.gpsimd if mode == "pool" else nc.vector
        adde = nc.gpsimd if mode in ("pool", "addpool", "t2addpool") else nc.vector
        t2 = tmp_pool.tile([KDIM, n], dt.float32, name=f"t2{c}", tag=f"t2{c}")
        if mode == "t2split":
            hn = n // 2
            nc.gpsimd.tensor_mul(t2[:, 0:hn], s_f[:, 0:hn],
                                 c_t[c][:, 0:hn])
            nc.vector.tensor_mul(t2[:, hn:n], s_f[:, hn:n],
                                 c_t[c][:, hn:n])
        else:
            t2e.tensor_mul(t2[:, :], s_f, c_t[c][:, :])
        t1 = tmp_pool.tile([KDIM, n], dt.float32, name=f"t1{c}", tag=f"t1{c}")
        t1e.scalar_tensor_tensor(
            out=t1[:, :], in0=s_g, scalar=0.5, in1=s_i,
            op0=OP.subtract, op1=OP.mult)
        adde.tensor_add(c_t[c][:, :], t1[:, :], t2[:, :])

    def tanh_chain(c):
        n = splits[c]
        if c == 0 and tc0_t is not None:
            tch = tc0_t
        else:
            tch = tmp_pool.tile([KDIM, n], dt.float32,
                                name=f"tc{c}", tag=f"tc{c}")
        nc.scalar.activation(tch[:, :], c_t[c][:, :], AF.Tanh, scale=2.0)
        return tch

    def h_chain(c, zs, tch, t):
        n = splits[c]
        ho = hoffs[c]
        mode = h_on[c % len(h_on)]
        eng = nc.gpsimd if mode == "pool" else nc.vector
        if mode == "split":
            # f-half first: unlocks the 4 f-dir h-matmuls of step t+1 early
            nc.vector.tensor_mul(ht_tiles[t][UP, ho:ho + n],
                                 zs[UP, 2 * n:3 * n], tch[UP, :])
            nc.vector.tensor_mul(ht_tiles[t][LO, ho:ho + n],
                                 zs[LO, 2 * n:3 * n], tch[LO, :])
        else:
            eng.tensor_mul(ht_tiles[t][:, ho:ho + n], zs[:, 2 * n:3 * n],
                           tch[:, :])

    for t0 in range(min(prefetch, l_steps)):
        new_x(t0)
    for c in range(K):
        x_mms(c, 0)

    for t in range(l_steps):
        if t + prefetch < l_steps:
            new_x(t + prefetch)
        ht_tiles[t] = ht_pool.tile([KDIM, HW_], dt.bfloat16,
                                   name="ht", tag="ht")
        zs_l = []
        if EMIT_ORDER == 0:
            for c in range(K):
                z = h_mms(c, t) if t > 0 else z_tiles.pop((c, 0))
                zs_l.append(sig_chain(c, z))
                tail_chain(c, zs_l[c])
            for c in range(K):
                tch = tanh_chain(c)
                h_chain(c, zs_l[c], tch, t)
        else:
            # tanh_c emitted after sig_{c+1}: ACT order s0 s1 t0 s2 t1 t2
            for c in range(K):
                z = h_mms(c, t) if t > 0 else z_tiles.pop((c, 0))
                zs_l.append(sig_chain(c, z))
                tail_chain(c, zs_l[c])
                if c >= 1:
                    tch = tanh_chain(c - 1)
                    h_chain(c - 1, zs_l[c - 1], tch, t)
            for c in range(K - 1, K):
                tch = tanh_chain(c)
                h_chain(c, zs_l[c], tch, t)
        if t + 1 < l_steps:
            for c in range(K):
                x_mms(c, t + 1)
        nc.sync.dma_start(out=o_d.ap()[:, t, :], in_=ht_tiles[t][:, :])
        if t >= 2:
            del ht_tiles[t - 2]
        if t in xt_tiles:
            del xt_tiles[t]


def _get_nc(l_steps, **kw):
    key = (l_steps, tuple(sorted(kw.items())))
    if key not in _CACHE:
        _CACHE[key] = _build(l_steps, **kw)
    return _CACHE[key]


def _prep_w(Wk, Wr):
    """[128, 256] contiguous: rows = [x-proj; h-proj], Keras gate col order
    i,f,g,o kept as-is; g cols pre-scaled by 2 (tanh via sigmoid)."""
    Wcat = np.concatenate([np.asarray(Wk, np.float32),
                           np.asarray(Wr, np.float32)], axis=0).copy()
    Wcat[:, 128:192] *= 2.0
    return Wcat


def kernel(ids, embed_table, Wk_f, Wr_f, b_f, Wk_b, Wr_b, b_b):
    import ml_dtypes
    from concourse import bass_utils

    bf16 = ml_dtypes.bfloat16
    assert not np.any(np.asarray(b_f)) and not np.any(np.asarray(b_b)), \
        "nonzero LSTM bias not supported by this kernel build"

    ids = np.asarray(ids)
    emb = np.asarray(embed_table, dtype=np.float32)
    wf = _prep_w(Wk_f, Wr_f).astype(bf16)
    wb = _prep_w(Wk_b, Wr_b).astype(bf16)

    nc = _get_nc(L)
    splits = SPLITS
    offs = [2 * sum(splits[:c]) for c in range(len(splits))]
    W = 2 * sum(splits)

    ids2 = ids.reshape(NSEQ, L)
    in_maps = []
    for m in range(NCORES):
        idc = ids2[m * NC_:(m + 1) * NC_]            # [NC_, L]
        xc = emb[idc]                                # [NC_, L, E] f32
        xT = xc.transpose(2, 1, 0)                   # [E, L, NC_] view
        xcat = np.empty((E, L, W), dtype=np.float32)
        s0 = 0
        for c, n in enumerate(splits):
            o = offs[c]
            xcat[:, :, o:o + n] = xT[:, :, s0:s0 + n]
            xcat[:, :, o + n:o + 2 * n] = xT[:, ::-1, s0:s0 + n]
            s0 += n
        wx = np.concatenate([wf[0:E, :], wb[0:E, :]], axis=1)
        wh = np.concatenate([wf[E:KDIM, :], wb[E:KDIM, :]], axis=0)
        im = {"x": xcat.astype(bf16), "wx": wx, "wh": wh}
        in_maps.append(im)

    res = bass_utils.run_bass_kernel_spmd(nc, in_maps,
                                          core_ids=list(range(NCORES)))

    hoffs = [sum(splits[:c]) for c in range(len(splits))]
    out = np.empty((NSEQ, L, 2 * H), dtype=np.float32)
    for m in range(NCORES):
        o_arr = np.asarray(res.results[m]["o"], dtype=np.float32)  # [128,L,256]
        s0 = 0
        for c, n in enumerate(splits):
            ho = hoffs[c]
            sl = slice(m * NC_ + s0, m * NC_ + s0 + n)
            hf = o_arr[0:H, :, ho:ho + n]              # [H, L, n]
            hb = o_arr[H:2 * H, ::-1, ho:ho + n]       # iteration -> time
            out[sl, :, 0:H] = hf.transpose(2, 1, 0)
            out[sl, :, H:2 * H] = hb.transpose(2, 1, 0)
            s0 += n
    return out.reshape(B, S, L, 2 * H)


# revision 35
# speedup vs baseline: 1.0023x; 1.0004x over previous
"""Bidirectional LSTM over embedded event ids — Trainium2 Bass kernel.

Problem shapes (hardcoded): ids [32,64,256] int32, embed [6000,64],
per-direction LSTM E=H=64, output [32,64,256,128] f32.

Strategy: pure data parallel over the flattened B*S=2048 sequence axis
(256 sequences per core on 8 cores).  The per-core 256 sequences are split
into 3 independent pipeline chains (78/90/88) so each chain's serial
per-step dependency path (h-matmuls -> sigmoid -> cell update -> tanh -> h)
hides under the other chains' engine work.  Both directions of one chain
share every instruction.

Per (chain, step), n = chain's sequence count, all tiles dir-stacked
(dir f rows 0:64, dir b rows 64:128), blocks [i | f | o | g] of n cols:
  z   [128,4n] PSUM f32 <- 8 x-matmuls (ctr 64, run a step ahead, PSUM
                           accumulation start) + 8 h-matmuls (ctr 64,
                           accumulate; rhs read straight from the stacked
                           h tile -- no rhs assembly on the critical path)
  zs  [128,4n] f32      <- ONE Sigmoid (g-gate weights pre-scaled by 2:
                           tanh(z) = 2*sig(2z)-1; zs must stay f32 -- a
                           bf16 s_g destroys g's precision near 0)
  t2  = s_f * c           tensor_tensor (DVE for chain 0, GPSIMD for
                           chains 1-2 to break the DVE serialization)
  t1  = (s_g-0.5)*s_i     fused scalar_tensor_tensor
  c   = t1 + t2           c stored as c/2; the x2 folds into Tanh's scale
  tc  = tanh(2*c)         ONE [128,n] Tanh
  h   = s_o * tc          ONE tensor_tensor, bf16, into the shared h tile;
                           the output DMA reads the same bytes (one in-DMA
                           + one out-DMA per step).

The backward direction reads x time-reversed via host-side layout.
bf16: matmul inputs (x, h, weights) and the output; f32: PSUM, gate
sigmoids, cell state.
"""

import numpy as np

EMIT_ORDER = 0
ZSB = 3
TMB = 6
PSUM_C0 = False

B, S, L, E, H, V = 32, 64, 256, 64, 64, 6000
NCORES = 8
NSEQ = B * S
NC_ = NSEQ // NCORES      # 256 sequences per core
GATES = 4 * H             # 256
KDIM = E + H              # 128

SPLITS = (78, 90, 88)     # sequences per chain (sum = NC_)

_CACHE = {}


def _build(l_steps, splits=SPLITS, prefetch=4, c_dtype="f32",
           tail_on=("vector", "t2pool", "t2pool"),
           h_on=("vector", "vector", "vector")):
    import concourse.bacc as bacc
    import concourse.tile as tile
    from concourse import mybir

    dt = mybir.dt
    AF = mybir.ActivationFunctionType
    OP = mybir.AluOpType

    K = len(splits)
    offs = [2 * sum(splits[:c]) for c in range(K)]   # x col offset per chain
    hoffs = [sum(splits[:c]) for c in range(K)]      # h col offset per chain
    W = 2 * sum(splits)                              # 512 x cols
    HW_ = sum(splits)                                # 256 h cols

    nc = bacc.Bacc("TRN2", num_devices=NCORES, debug=False)
    x_d = nc.dram_tensor("x", (E, l_steps, W), dt.bfloat16,
                         kind="ExternalInput")
    wx_d = nc.dram_tensor("wx", (E, 2 * GATES), dt.bfloat16,
                          kind="ExternalInput")
    wh_d = nc.dram_tensor("wh", (KDIM, GATES), dt.bfloat16,
                          kind="ExternalInput")
    o_d = nc.dram_tensor("o", (KDIM, l_steps, HW_), dt.bfloat16,
                         kind="ExternalOutput")

    cdt = dt.float32 if c_dtype == "f32" else dt.bfloat16

    with tile.TileContext(nc) as tc:
        with (
            tc.tile_pool(name="singles", bufs=1) as singles,
            tc.tile_pool(name="xt", bufs=prefetch + 3) as xt_pool,
            tc.tile_pool(name="ht", bufs=4) as ht_pool,
            tc.tile_pool(name="zs", bufs=ZSB * K) as zs_pool,
            tc.tile_pool(name="tmp", bufs=TMB * K) as tmp_pool,
        ):
            psum_pools = []
            import contextlib
            with contextlib.ExitStack() as stack:
                for c in range(K):
                    psum_pools.append(stack.enter_context(
                        tc.tile_pool(name=f"psum{c}", bufs=2, space="PSUM")))
                psum_singles = stack.enter_context(
                    tc.tile_pool(name="psum_s", bufs=1, space="PSUM")) \
                    if PSUM_C0 else None
                _body(nc, tc, dt, AF, OP, K, splits, offs, hoffs, W, HW_,
                      l_steps, prefetch, cdt, singles, xt_pool, ht_pool,
                      zs_pool, tmp_pool, psum_pools, x_d, wx_d, wh_d, o_d,
                      tail_on, h_on, psum_singles)

    nc.compile()
    return nc


def _body(nc, tc, dt, AF, OP, K, splits, offs, hoffs, W, HW_, l_steps,
          prefetch, cdt, singles, xt_pool, ht_pool, zs_pool, tmp_pool,
          psum_pools, x_d, wx_d, wh_d, o_d, tail_on=("vector",) * 8,
          h_on=("vector",) * 8, psum_singles=None):
    UP, LO = slice(0, 64), slice(64, 128)

    # weights host-packed into two tensors (2 DMAs instead of 4): wx holds
    # both dirs' x-projections side by side at base partition 0 (match x
    # tiles); wh holds Wh_f at partitions 0:64 and Wh_b at 64:128, where
    # their rhs (the stacked h tile) lives -- matmul requires lhsT and rhs
    # to share the base partition.
    wx_t = singles.tile([E, 2 * GATES], dt.bfloat16, name="wx", tag="wx")
    wh_t = singles.tile([KDIM, GATES], dt.bfloat16, name="wh", tag="wh")
    nc.sync.dma_start(out=wx_t[:, :], in_=wx_d.ap())
    # chain 0's cell state + tanh output can live in PSUM (its tail is
    # all-DVE, and DVE may have one PSUM operand per op): cuts the ACT
    # access latency on the wrap-critical tanh0 -> h0 path.
    c_t = []
    tc0_t = None
    for c in range(K):
        pool = psum_singles if (psum_singles is not None and c == 0) \
            else singles
        ct = pool.tile([KDIM, splits[c]], cdt, name=f"c{c}", tag=f"c{c}")
        nc.vector.memset(ct[:, :], 0.0)
        c_t.append(ct)
    if psum_singles is not None:
        tc0_t = psum_singles.tile([KDIM, splits[0]], dt.float32,
                                  name="tc0s", tag="tc0s")

    xt_tiles = {}
    ht_tiles = {}
    z_tiles = {}

    def new_x(t):
        tl = xt_pool.tile([E, W], dt.bfloat16, name="xt", tag="xt")
        xt_tiles[t] = tl
        nc.sync.dma_start(out=tl[:, :], in_=x_d.ap()[:, t, :])

    # per-chain z layout [128, 4n]: blocks [i | f | o | g] of n cols, each
    # dir-stacked (f rows 0:64, b rows 64:128).  Keras gate cols in w:
    # i 0:64, f 64:128, g 128:192 (x2), o 192:256.
    GCOL = {"i": slice(0, 64), "f": slice(64, 128),
            "o": slice(192, 256), "g": slice(128, 192)}
    ORDER = ("i", "f", "o", "g")

    def x_mms(c, t):
        # x-projection, runs ahead of the recurrence (start of the psum
        # accumulation group; t=0 also closes it -- no h contribution)
        n = splits[c]
        o = offs[c]
        z = psum_pools[c].tile([128, 4 * n], dt.float32,
                               name=f"z{c}", tag=f"z{c}")
        z_tiles[(c, t)] = z
        xf = xt_tiles[t][:, o:o + n]
        xb = xt_tiles[t][:, o + n:o + 2 * n]
        last = t == 0
        for k, gate in enumerate(ORDER):
            g = GCOL[gate]
            nc.tensor.matmul(z[UP, k * n:(k + 1) * n],
                             wx_t[:, g], xf, start=True, stop=last)
            nc.tensor.matmul(z[LO, k * n:(k + 1) * n],
                             wx_t[:, g.start + GATES:g.stop + GATES], xb,
                             start=True, stop=last)
        return z

    def h_mms(c, t):
        # recurrent part: contraction 64 over h(t-1), read straight from the
        # dir-stacked h tile (no rhs assembly on the critical path)
        n = splits[c]
        ho = hoffs[c]
        z = z_tiles.pop((c, t))
        hprev = ht_tiles[t - 1]
        hf = hprev[UP, ho:ho + n]
        hb = hprev[LO, ho:ho + n]
        for k, gate in enumerate(ORDER):
            nc.tensor.matmul(z[UP, k * n:(k + 1) * n],
                             wh_t[UP, GCOL[gate]], hf,
                             start=False, stop=True)
        for k, gate in enumerate(ORDER):
            nc.tensor.matmul(z[LO, k * n:(k + 1) * n],
                             wh_t[LO, GCOL[gate]], hb,
                             start=False, stop=True)
        return z

    def sig_chain(c, z):
        n = splits[c]
        zs = zs_pool.tile([128, 4 * n], dt.float32,
                          name=f"zs{c}", tag=f"zs{c}")
        nc.scalar.activation(zs[:, :], z[:, :], AF.Sigmoid)
        return zs

    def tail_chain(c, zs):
        # c_new = s_f*c + 2*(s_i*(s_g-0.5)); s_* stay f32 (a bf16 s_g makes
        # 2*s_g-1 lose all precision near g=0).  All [128, n] dir-stacked.
        n = splits[c]
        s_i = zs[:, 0:n]
        s_f = zs[:, n:2 * n]
        s_g = zs[:, 3 * n:4 * n]
        # the cell state is stored as c/2 so the combine is a plain add;
        # tanh reads it with scale=2 (free input affine on ACT)
        mode = tail_on[c % len(tail_on)]
        t2e = nc.gpsimd if mode in ("pool", "t2pool", "t2addpool") else nc.vector
        t1e = nc.gpsimd if mode == "pool" else nc.vector
        adde = nc.gpsimd if mode in ("pool", "addpool", "t2addpool") else nc.vector
        t2 = tmp_pool.tile([KDIM, n], dt.float32, name=f"t2{c}", tag=f"t2{c}")
        if mode == "t2split":
            hn = n // 2
            nc.gpsimd.tensor_mul(t2[:, 0:hn], s_f[:, 0:hn],
                                 c_t[c][:, 0:hn])
            nc.vector.tensor_mul(t2[:, hn:n], s_f[:, hn:n],
                                 c_t[c][:, hn:n])
        else:
            t2e.tensor_mul(t2[:, :], s_f, c_t[c][:, :])
        t1 = tmp_pool.tile([KDIM, n], dt.float32, name=f"t1{c}", tag=f"t1{c}")
        t1e.scalar_tensor_tensor(
            out=t1[:, :], in0=s_g, scalar=0.5, in1=s_i,
            op0=OP.subtract, op1=OP.mult)
        adde.tensor_add(c_t[c][:, :], t1[:, :], t2[:, :])

    def tanh_chain(c):
        n = splits[c]
        if c == 0 and tc0_t is not None:
            tch = tc0_t
        else:
            tch = tmp_pool.tile([KDIM, n], dt.float32,
                                name=f"tc{c}", tag=f"tc{c}")
        nc.scalar.activation(tch[:, :], c_t[c][:, :], AF.Tanh, scale=2.0)
        return tch

    def h_chain(c, zs, tch, t):
        n = splits[c]
        ho = hoffs[c]
        mode = h_on[c % len(h_on)]
        eng = nc.gpsimd if mode == "pool" else nc.vector
        if mode == "split":
            # f-half first: unlocks the 4 f-dir h-matmuls of step t+1 early
            nc.vector.tensor_mul(ht_tiles[t][UP, ho:ho + n],
                                 zs[UP, 2 * n:3 * n], tch[UP, :])
            nc.vector.tensor_mul(ht_tiles[t][LO, ho:ho + n],
                                 zs[LO, 2 * n:3 * n], tch[LO, :])
        else:
            eng.tensor_mul(ht_tiles[t][:, ho:ho + n], zs[:, 2 * n:3 * n],
                           tch[:, :])

    # x(0) issued right after the x-weights; wh is not needed until the
    # first h-matmuls (step 1), so its DMA goes after x(0) in the SP queue
    new_x(0)
    nc.sync.dma_start(out=wh_t[:, :], in_=wh_d.ap())
    for t0 in range(1, min(prefetch, l_steps)):
        new_x(t0)
    for c in range(K):
        x_mms(c, 0)

    for t in range(l_steps):
        if t + prefetch < l_steps:
            new_x(t + prefetch)
        ht_tiles[t] = ht_pool.tile([KDIM, HW_], dt.bfloat16,
                                   name="ht", tag="ht")
        zs_l = []
        if EMIT_ORDER == 0:
            for c in range(K):
                z = h_mms(c, t) if t > 0 else z_tiles.pop((c, 0))
                zs_l.append(sig_chain(c, z))
                tail_chain(c, zs_l[c])
            for c in range(K):
                tch = tanh_chain(c)
                h_chain(c, zs_l[c], tch, t)
        else:
            # tanh_c emitted after sig_{c+1}: ACT order s0 s1 t0 s2 t1 t2
            for c in range(K):
                z = h_mms(c, t) if t > 0 else z_tiles.pop((c, 0))
                zs_l.append(sig_chain(c, z))
                tail_chain(c, zs_l[c])
                if c >= 1:
                    tch = tanh_chain(c - 1)
                    h_chain(c - 1, zs_l[c - 1], tch, t)
            for c in range(K - 1, K):
                tch = tanh_chain(c)
                h_chain(c, zs_l[c], tch, t)
        if t + 1 < l_steps:
            for c in range(K):
                x_mms(c, t + 1)
        nc.sync.dma_start(out=o_d.ap()[:, t, :], in_=ht_tiles[t][:, :])
        if t >= 2:
            del ht_tiles[t - 2]
        if t in xt_tiles:
            del xt_tiles[t]


def _get_nc(l_steps, **kw):
    key = (l_steps, tuple(sorted(kw.items())))
    if key not in _CACHE:
        _CACHE[key] = _build(l_steps, **kw)
    return _CACHE[key]


def _prep_w(Wk, Wr):
    """[128, 256] contiguous: rows = [x-proj; h-proj], Keras gate col order
    i,f,g,o kept as-is; g cols pre-scaled by 2 (tanh via sigmoid)."""
    Wcat = np.concatenate([np.asarray(Wk, np.float32),
                           np.asarray(Wr, np.float32)], axis=0).copy()
    Wcat[:, 128:192] *= 2.0
    return Wcat


def kernel(ids, embed_table, Wk_f, Wr_f, b_f, Wk_b, Wr_b, b_b):
    import ml_dtypes
    from concourse import bass_utils

    bf16 = ml_dtypes.bfloat16
    assert not np.any(np.asarray(b_f)) and not np.any(np.asarray(b_b)), \
        "nonzero LSTM bias not supported by this kernel build"

    ids = np.asarray(ids)
    emb = np.asarray(embed_table, dtype=np.float32)
    wf = _prep_w(Wk_f, Wr_f).astype(bf16)
    wb = _prep_w(Wk_b, Wr_b).astype(bf16)

    nc = _get_nc(L)
    splits = SPLITS
    offs = [2 * sum(splits[:c]) for c in range(len(splits))]
    W = 2 * sum(splits)

    ids2 = ids.reshape(NSEQ, L)
    in_maps = []
    for m in range(NCORES):
        idc = ids2[m * NC_:(m + 1) * NC_]            # [NC_, L]
        xc = emb[idc]                                # [NC_, L, E] f32
        xT = xc.transpose(2, 1, 0)                   # [E, L, NC_] view
        xcat = np.empty((E, L, W), dtype=np.float32)
        s0 = 0
        for c, n in enumerate(splits):
            o = offs[c]
            xcat[:, :, o:o + n] = xT[:, :, s0:s0 + n]
            xcat[:, :, o + n:o + 2 * n] = xT[:, ::-1, s0:s0 + n]
            s0 += n
        wx = np.concatenate([wf[0:E, :], wb[0:E, :]], axis=1)
        wh = np.concatenate([wf[E:KDIM, :], wb[E:KDIM, :]], axis=0)
        im = {"x": xcat.astype(bf16), "wx": wx, "wh": wh}
        in_maps.append(im)

    res = bass_utils.run_bass_kernel_spmd(nc, in_maps,
                                          core_ids=list(range(NCORES)))

    hoffs = [sum(splits[:c]) for c in range(len(splits))]
    out = np.empty((NSEQ, L, 2 * H), dtype=np.float32)
    for m in range(NCORES):
        o_arr = np.asarray(res.results[m]["o"], dtype=np.float32)  # [128,L,256]
        s0 = 0
        for c, n in enumerate(splits):
            ho = hoffs[c]
            sl = slice(m * NC_ + s0, m * NC_ + s0 + n)
            hf = o_arr[0:H, :, ho:ho + n]              # [H, L, n]
            hb = o_arr[H:2 * H, ::-1, ho:ho + n]       # iteration -> time
            out[sl, :, 0:H] = hf.transpose(2, 1, 0)
            out[sl, :, H:2 * H] = hb.transpose(2, 1, 0)
            s0 += n
    return out.reshape(B, S, L, 2 * H)
